# revision 13
# baseline (speedup 1.0000x reference)
"""nn_DWTFrontEnd Trainium2 Bass kernel (composed band ops, fp8 DoubleRow).

3-level db4 DWT band split/reconstruction of 2048 signals x 8192.
Sharding: 256 signals per core (8 cores), position-major SBUF layout
([128 positions, 256 signals] blocks).

Each band's analysis+synthesis is folded into one banded operator P_b
(8192x8192, support +-49 samples).  Output is computed in half-block-
SHIFTED 128-position tiles so each output tile needs exactly one PAIR
of 128-aligned input K-tiles -> one fp8 DoubleRow matmul contracts both
K-tiles at 0.5 cycles/row.  The head [0,64) and tail [8128,8192) edge
outputs fuse into a single DR entry (head weights in PSUM partitions
0..63, tail in 64..127, K-pair (x_blk0, x_blk63) via a stride-63 AP).
Precision is recovered with a 3-term expansion (weights scaled by SW=8
so the fp8 residual store stays out of the subnormal floor; the 1/SW
rescale rides the PSUM->SBUF copies):

    psum = Wh (x) xh  +  Wh (x) xl  +  Wl (x) xh        (all fp8e4 DR)
    y    = psum / SW                                     (copy w/ scale)

with xh = fp8(x), xl = fp8(x - xh) split on the host (same input bytes
as fp16), Wh = fp8(SW*W), Wl = fp8(SW*W - Wh).  Weight blocks dedupe to
8 per band (Toeplitz) -> 512 KiB.  Verified vs reference: rel ~3e-3.
DVE/ACT split the scaled copies; all DMAs issue from the SP queue; the
edge entries run mid-stream (batched 4-band head/tail DMAs) so the run
ends on four clean full-size group DMAs; PSUM uses all 8 banks so the
PE never stalls.  TimelineSim: DMA-device busy 59.7us with zero idle
(w 1.5 + x 11.6 + y 46.6 at the modeled 360 GB/s), 63.5us total.
"""
import sys
for p in ("/opt/trn_rl_repo", "/root/.axon_site/_ro/trn_rl_repo"):
    if p not in sys.path:
        sys.path.append(p)

import numpy as np
import ml_dtypes
import concourse.bass as bass
import concourse.mybir as mybir
import concourse.tile as tile
from concourse.tile_rust import add_dep_helper

F32 = mybir.dt.float32
F16 = mybir.dt.float16
F8 = mybir.dt.float8e4
E4 = ml_dtypes.float8_e4m3
DR = mybir.MatmulPerfMode.DoubleRow

REC_LO = np.array([0.23037781330885523, 0.7148465705525415, 0.6308807679295904,
                   -0.027983769416983849, -0.18703481171888114, 0.030841381835986965,
                   0.032883011666982945, -0.010597401784997278], dtype=np.float64)
F = 8
REC_HI = np.array([(-1.0) ** k * REC_LO[F - 1 - k] for k in range(F)], dtype=np.float64)
DEC_LO = REC_LO[::-1].copy()
DEC_HI = REC_HI[::-1].copy()

N = 8192
L1, L2, L3 = 4099, 2053, 1030
S = 256            # signals per core
P = 128            # positions per block
NBLK = N // P      # 64 input blocks
SW = 8.0           # weight pre-scale (undone in the copy)


def _dwt_apply(X, filt):
    n = X.shape[1]
    idx = np.pad(np.arange(n), (F - 1, F - 1), mode='symmetric')[1:]
    Xe = X[:, idx]
    L = (n + 13 - F) // 2 + 1
    out = np.zeros((X.shape[0], L), dtype=X.dtype)
    for k in range(F):
        out += filt[k] * Xe[:, k:k + 2 * (L - 1) + 1:2]
    return out


def _idwt_half_apply(A, filt):
    B, L = A.shape
    n_out = 2 * L - 6
    out = np.zeros((B, n_out), dtype=A.dtype)
    j = np.arange(L)
    for k in range(F):
        n = 2 * j - k + 1
        valid = (n >= 0) & (n < n_out)
        out[:, n[valid]] += filt[k] * A[:, j[valid]]
    return out


def _synth_chain(I, steps):
    A = I
    for filt, trim_to in steps:
        A = _idwt_half_apply(A, filt)
        if trim_to is not None and A.shape[1] == trim_to + 1:
            A = A[:, :trim_to]
    return A


def _band_operator(b):
    """P[r, m] = d y_b[m] / d x[r], shape (N, N), f32."""
    f32 = np.float32
    if b == 0:
        T = _synth_chain(np.eye(L3, dtype=f32), [(DEC_LO, L2), (DEC_LO, L1),
                                                 (DEC_LO, None)])[:, :N]
        M = _dwt_apply(np.eye(L2, dtype=f32), REC_LO) @ T
        M = _dwt_apply(np.eye(L1, dtype=f32), REC_LO) @ M
        M = _dwt_apply(np.eye(N, dtype=f32), REC_LO) @ M
    elif b == 1:
        T = _synth_chain(np.eye(L3, dtype=f32), [(DEC_HI, L2), (DEC_LO, L1),
                                                 (DEC_LO, None)])[:, :N]
        M = _dwt_apply(np.eye(L2, dtype=f32), REC_HI) @ T
        M = _dwt_apply(np.eye(L1, dtype=f32), REC_LO) @ M
        M = _dwt_apply(np.eye(N, dtype=f32), REC_LO) @ M
    elif b == 2:
        T = _synth_chain(np.eye(L2, dtype=f32), [(DEC_HI, L1), (DEC_LO, None)])[:, :N]
        M = _dwt_apply(np.eye(L1, dtype=f32), REC_HI) @ T
        M = _dwt_apply(np.eye(N, dtype=f32), REC_LO) @ M
    else:
        T = _synth_chain(np.eye(L1, dtype=f32), [(DEC_HI, None)])[:, :N]
        M = _dwt_apply(np.eye(N, dtype=f32), REC_HI) @ T
    return M


def _q8(a):
    return a.astype(E4).astype(np.float32)


def _build_plan():
    """Weight blocks per band b (8 each, [128,128] fp8-valued f32):
    base+0: Lh  base+1: Rh  base+2: Ll  base+3: Rl   (interior pair)
    base+4: Hh  base+5: Hl  (head, out cols 0..63; cols 64..127 zero)
    base+6: Th' base+7: Tl' (tail, out cols 64..127; cols 0..63 zero)
    Entry e=1..63 of band b covers y[64+128(e-1) : 64+128e) from K-pair
    (e-1, e); the fused entry covers y[0:64)+y[8128:8192) from K-pair
    (0, 63) with a stride-63 rhs slice."""
    blocks = []
    for b in range(4):
        Pb = _band_operator(b)
        H = np.zeros((P, P), np.float32)
        H[:, :64] = SW * Pb[0:P, 0:64]
        L = SW * Pb[0:P, 64:192]
        R = SW * Pb[P:2 * P, 64:192]
        T = np.zeros((P, P), np.float32)
        T[:, 64:] = SW * Pb[63 * P:64 * P, N - 64:N]
        assert np.abs(Pb[2 * P:, 0:64]).max() < 1e-8
        assert np.abs(Pb[:61 * P, N - 64:N]).max() < 1e-8
        for e in range(1, 64):
            lo = 64 + P * (e - 1)
            assert np.allclose(SW * Pb[(e - 1) * P:e * P, lo:lo + P], L,
                               rtol=0, atol=1e-4)
            assert np.allclose(SW * Pb[e * P:(e + 1) * P, lo:lo + P], R,
                               rtol=0, atol=1e-4)
            mask = np.ones(N, dtype=bool)
            mask[(e - 1) * P:(e + 1) * P] = False
            assert np.abs(Pb[mask][:, lo:lo + P]).max() < 1e-8
        Lh, Rh, Hh, Th = _q8(L), _q8(R), _q8(H), _q8(T)
        blocks += [Lh, Rh, _q8(L - Lh), _q8(R - Rh),
                   Hh, _q8(H - Hh), Th, _q8(T - Th)]
    return np.stack(blocks)  # [32, 128, 128]


_PLAN = None


def _get_plan():
    global _PLAN
    if _PLAN is None:
        _PLAN = _build_plan()
    return _PLAN


def build_kernel():
    wblocks = _get_plan()
    nB = wblocks.shape[0]
    wflat = np.ascontiguousarray(
        wblocks.transpose(1, 0, 2).reshape(P, nB * P)).astype(E4)

    nc = bass.Bass(trn_type="TRN2")
    # x8: [position, {hi,lo}, signal] so the innermost run stays 512B
    x_d = nc.dram_tensor("x", [N, 2, S], F8, kind="ExternalInput").ap()
    y_d = nc.dram_tensor("y", [4, N, S], F16, kind="ExternalOutput").ap()
    w_d = nc.inline_tensor(wflat, name="wts").ap()

    sinks = []
    eng_last = {}
    eng_cost = {'vector': 0.0, 'scalar': 0.0}
    eng_rate = {'vector': 1.05, 'scalar': 0.85}
    eng_fix = {'vector': 170.0, 'scalar': 220.0}
    inv = 1.0 / SW

    with tile.TileContext(nc) as tc:
        with tc.tile_pool(name="ded", bufs=1) as ded, \
             tc.tile_pool(name="stg", bufs=6) as stg, \
             tc.tile_pool(name="psA", bufs=4, space="PSUM") as psA:

            wsb = ded.tile([P, nB * P], F8, tag="wsb")
            sinks.append(nc.sync.dma_start(wsb, w_d))
            w3 = wsb.rearrange("p (n f) -> p n f", f=P)

            def wpair(i, j):
                return w3[:, i:j + 1:j - i, :] if j - i > 1 else w3[:, i:i + 2, :]

            x_t = ded.tile([P, NBLK * 2 * S], F8, tag="xt", name="xt")
            x4 = x_t.rearrange("p (b hl s) -> p b hl s", hl=2, s=S)

            def xpair(k0, k1, hl):
                if k1 - k0 > 1:
                    return x4[:, k0:k1 + 1:k1 - k0, hl]
                return x4[:, k0:k0 + 2, hl]

            def load_x(b0, b1):
                src = x_d[b0 * P:b1 * P, :, :].rearrange(
                    "(b p) hl s -> p b hl s", p=P)
                sinks.append(nc.sync.dma_start(x4[:, b0:b1], src))

            load_x(0, 2)       # small first chunks so PE starts early
            load_x(2, 9)
            load_x(9, 16)
            for g in range(2, 8):
                load_x(8 * g, 8 * g + 8)

            def pick_engine():
                return min(eng_cost, key=eng_cost.get)

            def do_copy(dst_ap, src_ap, width, force=None):
                e = force or pick_engine()
                eng_cost[e] += width * eng_rate[e] + eng_fix[e]
                if e == 'vector':
                    eng_last[e] = nc.vector.tensor_scalar_mul(dst_ap, src_ap, inv)
                else:
                    eng_last[e] = nc.scalar.mul(dst_ap, src_ap, inv)

            def edge_group():
                """Fused head+tail entry for all 4 bands: one psum tile,
                one copy, 8 small DMAs.  Runs mid-stream (after the x load
                lands) so its per-DMA issue overheads hide under backlog."""
                stage = stg.tile([P, 4 * S], F16, tag="stg_e", name="stg_e")
                ps = psA.tile([P, 4 * S], F32, tag="psA", name="psA")
                for b in range(4):
                    base = 8 * b
                    terms = [(wpair(base + 4, base + 6), 0),
                             (wpair(base + 4, base + 6), 1),
                             (wpair(base + 5, base + 7), 0)]
                    for q, (wp, hl) in enumerate(terms):
                        nc.tensor.matmul(
                            ps[:, b * S:(b + 1) * S],
                            wp, xpair(0, NBLK - 1, hl),
                            start=(q == 0), stop=(q == len(terms) - 1),
                            perf_mode=DR)
                do_copy(stage, ps, 4 * S)
                # one DMA for all 4 bands' heads, one for all tails
                dsth = y_d[:, 0:64, :].rearrange("b p s -> p b s")
                srch = stage[0:64, :].rearrange("p (b s) -> p b s", s=S)
                sinks.append(nc.sync.dma_start(dsth, srch))
                dstt = y_d[:, N - 64:N, :].rearrange("b p s -> p b s")
                srct = stage[64:P, :].rearrange("p (b s) -> p b s", s=S)
                sinks.append(nc.sync.dma_start(dstt, srct))

            for g in range(8):
                # g<7: entries 1+8g .. 8+8g; g==7: entries 57..63
                es = list(range(1 + 8 * g, min(9 + 8 * g, NBLK)))
                ns = len(es)
                for b in range(4):
                    base = 8 * b
                    stage = stg.tile([P, 8 * S], F16, tag="stg", name="stg")
                    last = (g == 7 and b == 3)
                    for t in range(0, ns, 4):
                        sub = es[t:t + 4]
                        ps = psA.tile([P, 4 * S], F32, tag="psA", name="psA")
                        for h, e in enumerate(sub):
                            terms = [(wpair(base, base + 1), 0),
                                     (wpair(base, base + 1), 1),
                                     (wpair(base + 2, base + 3), 0)]
                            for q, (wp, hl) in enumerate(terms):
                                nc.tensor.matmul(
                                    ps[:, h * S:(h + 1) * S],
                                    wp, xpair(e - 1, e, hl),
                                    start=(q == 0), stop=(q == len(terms) - 1),
                                    perf_mode=DR)
                        w = len(sub)
                        if last:
                            # split the final copies across both engines to
                            # shorten the end-of-run critical path
                            hw_ = w // 2
                            do_copy(stage[:, t * S:(t + hw_) * S],
                                    ps[:, 0:hw_ * S], hw_ * S, force='vector')
                            do_copy(stage[:, (t + hw_) * S:(t + w) * S],
                                    ps[:, hw_ * S:w * S], (w - hw_) * S,
                                    force='scalar')
                        else:
                            do_copy(stage[:, t * S:(t + w) * S],
                                    ps[:, 0:w * S], w * S)
                    # DMA this group's output
                    lo = 64 + 1024 * g
                    dst = y_d[b, lo:lo + ns * P, :].rearrange(
                        "(k p) s -> p k s", p=P)
                    src = stage[:, 0:ns * S].rearrange("p (k s) -> p k s", s=S)
                    sinks.append(nc.sync.dma_start(dst, src))
                if g == 2:
                    edge_group()

            tc.no_sync_barrier()
            for s2 in sinks + list(eng_last.values()):
                nn = nc.sync.nop()
                add_dep_helper(nn.ins, s2.ins, reason="tail absorb")
    # PE is hardware-decoded: a Matmult may carry at most one sync wait.
    # Move extra waits onto the paired (SW-decoded) Ldweights, then split
    # any remaining multi-wait instructions via event semaphores.
    import bass_rust
    bass_rust.move_matmul_waits_to_ldweights(nc.m)
    bass_rust.generate_event_semaphores(nc)
    return nc


_NC_CACHE = None


def run_full(x_full, trace=False):
    from concourse.bass_utils import run_bass_kernel_spmd
    global _NC_CACHE
    B, C, n = x_full.shape
    xf = np.ascontiguousarray(x_full.reshape(B * C, n).astype(np.float32))
    n_cores = 8
    if _NC_CACHE is None:
        _NC_CACHE = build_kernel()
    nc = _NC_CACHE
    in_maps = []
    for i in range(n_cores):
        shard = xf[i * S:(i + 1) * S].T                  # (8192, 256) f32
        xh = shard.astype(E4)
        xl = (shard - xh.astype(np.float32)).astype(E4)
        x8 = np.ascontiguousarray(
            np.stack([xh, xl], axis=1))                  # (8192, 2, 256) fp8
        in_maps.append({"x": x8})
    res = run_bass_kernel_spmd(nc, in_maps, core_ids=list(range(n_cores)),
                               trace=trace)
    bands = np.empty((4, B * C, n), dtype=np.float32)
    for i in range(n_cores):
        y = np.asarray(res.results[i]["y"])              # (4, 8192, 256) f16
        bands[:, i * S:(i + 1) * S, :] = y.transpose(0, 2, 1).astype(np.float32)
    out = tuple(bands[j].reshape(B, C, n) for j in range(4))
    return out, res


def kernel(x):
    out, _ = run_full(np.asarray(x))
    return out
